# revision 1
# baseline (speedup 1.0000x reference)
"""Trainium2 Bass kernel for CustomHyperSemanticMessagePassing.

Hypergraph multi-head attention message passing, N=4096 nodes, E=4096 edges,
DEG=CARD=8, D=256, H=8 heads. Sharding: data-parallel over nodes (512/core).

Host: derives edge_of_node / node_of_edge index lists from the binary
incidence matrix, pre-combines the small projection weights, and pre-shards
the pair tensors. Device (per core): builds K/V/EK tables with PE matmuls
(replicated), then per 128-node tile gathers per-pair K|V rows with indirect
DMA and runs the attention (scores, exp, weighted sum, out-proj, relu).

Key identities used:
  k_pair = Wh[u] @ Wk.T + We[e] @ Wk.T + bk  -> gather(K_tab)[u] + gather(EK_tab)[e]
  v_pair = Wh[u] @ Wv.T + bv                 -> gather(V_tab)[u]
  softmax without max-subtraction (scores are O(1) bounded), so per-round
  partial exp sums / weighted sums combine by plain addition.
"""
import numpy as np

import bass_rust
import orjson
import concourse.bass as bass
import concourse.tile as tile
import concourse.bass_utils as bass_utils
import concourse.bass2jax as bass2jax
from concourse import mybir
from concourse.masks import make_identity

F32 = mybir.dt.float32
BF16 = mybir.dt.bfloat16
I32 = mybir.dt.int32

N, E, D, EDGE_DIM = 4096, 4096, 256, 64
H, DH, DEG, CARD = 8, 32, 8, 8
L = DEG * CARD
NCORES = 8
NSH = N // NCORES          # nodes per core
NT = NSH // 128            # 128-node tiles per core


# ---------------------------------------------------------------------------
# walrus workaround: this build accepts only one sync-wait per instruction;
# split extras into injected single-wait NoOps at the BIR-JSON level.
_ORIG_COMPILE = bass_utils.compile_bir_kernel
_ctr = [0]


def _split_multiwaits(bir_json: bytes) -> bytes:
    bir = orjson.loads(bir_json)
    changed = False
    for f in bir.get("functions", []):
        for blk in f.get("blocks", []):
            out = []
            for ins in blk.get("instructions", []):
                si = ins.get("sync_info")
                waits = (si or {}).get("on_wait") or []
                if len(waits) > 1 and ins.get("engine") not in (None, "Unassigned"):
                    changed = True
                    for w in waits[:-1]:
                        _ctr[0] += 1
                        out.append({
                            "debug": ins.get("debug"),
                            "engine": ins["engine"],
                            "ins": [], "outs": [],
                            "name": f"WSPLIT-{_ctr[0]}",
                            "opcode": "NoOp",
                            "sync_info": {"on_wait": [w], "on_update": []},
                        })
                    si["on_wait"] = waits[-1:]
                out.append(ins)
            blk["instructions"] = out
    return orjson.dumps(bir) if changed else bir_json


def _patched_compile(bir_json, tmpdir, neff_name="file.neff"):
    return _ORIG_COMPILE(_split_multiwaits(bytes(bir_json)), tmpdir,
                         neff_name=neff_name)


def _install_patch():
    bass_utils.compile_bir_kernel = _patched_compile
    bass2jax.compile_bir_kernel = _patched_compile


_install_patch()


# ---------------------------------------------------------------------------
def build_nc():
    nc = bass.Bass(num_devices=NCORES)
    # replicated inputs
    xT = nc.declare_dram_parameter("xT", [D, N], BF16, isOutput=False)
    eaT = nc.declare_dram_parameter("eaT", [EDGE_DIM, E], BF16, isOutput=False)
    wkc = nc.declare_dram_parameter("wkc", [D, D], BF16, isOutput=False)
    wvc = nc.declare_dram_parameter("wvc", [D, D], BF16, isOutput=False)
    wqc = nc.declare_dram_parameter("wqc", [D, D], BF16, isOutput=False)
    wek = nc.declare_dram_parameter("wek", [EDGE_DIM, D], BF16, isOutput=False)
    owT = nc.declare_dram_parameter("owT", [D, D], F32, isOutput=False)
    bkv_b = nc.declare_dram_parameter("bkv_b", [128, 2 * D], F32, isOutput=False)
    bq_b = nc.declare_dram_parameter("bq_b", [128, D], F32, isOutput=False)
    bk_b = nc.declare_dram_parameter("bk_b", [128, D], F32, isOutput=False)
    bo_b = nc.declare_dram_parameter("bo_b", [128, D], F32, isOutput=False)
    # per-core inputs
    xT_own = nc.declare_dram_parameter("xT_own", [D, NSH], BF16, isOutput=False)
    pu = nc.declare_dram_parameter("pu", [NSH, L], I32, isOutput=False)
    pe = nc.declare_dram_parameter("pe", [NSH, DEG], I32, isOutput=False)
    # output
    out = nc.declare_dram_parameter("out", [NSH, D], F32, isOutput=True)
    # internal tables
    kv_tab = nc.dram_tensor("kv_tab", [N, 2 * D], BF16)
    ek_tab = nc.dram_tensor("ek_tab", [E, D], BF16)

    with tile.TileContext(nc) as tc, \
         tc.tile_pool(name="wpool", bufs=1) as wp, \
         tc.tile_pool(name="xpool", bufs=3) as xp, \
         tc.tile_pool(name="tpool", bufs=3) as tp, \
         tc.tile_pool(name="qpool", bufs=NT) as qp, \
         tc.tile_pool(name="gpool", bufs=3) as gp, \
         tc.tile_pool(name="apool", bufs=2) as ap_, \
         tc.tile_pool(name="cpool", bufs=2) as cp, \
         tc.tile_pool(name="psA", bufs=2, space="PSUM") as psA, \
         tc.tile_pool(name="psB", bufs=2, space="PSUM") as psB, \
         tc.tile_pool(name="psC", bufs=2, space="PSUM") as psC:

        # ---- load weights/biases (resident) ----
        wk_t = wp.tile([128, 2, D], BF16)
        nc.sync.dma_start(out=wk_t[:], in_=wkc[:].rearrange("(c k) o -> k c o", c=2))
        wv_t = wp.tile([128, 2, D], BF16)
        nc.sync.dma_start(out=wv_t[:], in_=wvc[:].rearrange("(c k) o -> k c o", c=2))
        wq_t = wp.tile([128, 2, D], BF16)
        nc.sync.dma_start(out=wq_t[:], in_=wqc[:].rearrange("(c k) o -> k c o", c=2))
        wek_t = wp.tile([EDGE_DIM, D], BF16)
        nc.sync.dma_start(out=wek_t[:], in_=wek[:])
        owT_t = wp.tile([128, 2, D], F32)
        nc.sync.dma_start(out=owT_t[:], in_=owT[:].rearrange("(c k) o -> k c o", c=2))
        bkv_t = wp.tile([128, 2 * D], F32)
        nc.sync.dma_start(out=bkv_t[:], in_=bkv_b[:])
        bq_t = wp.tile([128, D], F32)
        nc.sync.dma_start(out=bq_t[:], in_=bq_b[:])
        bk_t = wp.tile([128, D], F32)
        nc.sync.dma_start(out=bk_t[:], in_=bk_b[:])
        bo_t = wp.tile([128, D], F32)
        nc.sync.dma_start(out=bo_t[:], in_=bo_b[:])
        ident = wp.tile([128, 128], F32)
        make_identity(nc, ident[:])

        # ---- phase T: build KV table ----
        for m in range(N // 128):
            xt = xp.tile([128, 2, 128], BF16, tag="xt")
            nc.sync.dma_start(
                out=xt[:],
                in_=xT[:, bass.ts(m, 128)].rearrange("(c k) n -> k c n", c=2))
            pkv = psA.tile([128, 2 * D], F32, space="PSUM", tag="pkv")
            nc.tensor.matmul(out=pkv[:, 0:D], lhsT=xt[:, 0, :], rhs=wk_t[:, 0, :],
                             start=True, stop=False)
            nc.tensor.matmul(out=pkv[:, 0:D], lhsT=xt[:, 1, :], rhs=wk_t[:, 1, :],
                             start=False, stop=True)
            nc.tensor.matmul(out=pkv[:, D:2 * D], lhsT=xt[:, 0, :], rhs=wv_t[:, 0, :],
                             start=True, stop=False)
            nc.tensor.matmul(out=pkv[:, D:2 * D], lhsT=xt[:, 1, :], rhs=wv_t[:, 1, :],
                             start=False, stop=True)
            kv_sb = tp.tile([128, 2 * D], BF16, tag="kvsb")
            nc.vector.tensor_tensor(out=kv_sb[:], in0=pkv[:], in1=bkv_t[:],
                                    op=mybir.AluOpType.add)
            nc.sync.dma_start(out=kv_tab[bass.ts(m, 128), :], in_=kv_sb[:])

        # ---- phase T: build EK table ----
        for m in range(E // 128):
            et = xp.tile([EDGE_DIM, 128], BF16, tag="et")
            nc.sync.dma_start(out=et[:], in_=eaT[:, bass.ts(m, 128)])
            pek = psB.tile([128, D], F32, space="PSUM", tag="p256")
            nc.tensor.matmul(out=pek[:], lhsT=et[:], rhs=wek_t[:],
                             start=True, stop=True)
            ek_sb = tp.tile([128, D], BF16, tag="eksb")
            nc.vector.tensor_tensor(out=ek_sb[:], in0=pek[:], in1=bk_t[:],
                                    op=mybir.AluOpType.add)
            nc.sync.dma_start(out=ek_tab[bass.ts(m, 128), :], in_=ek_sb[:])

        # ---- phase T: q for own nodes (kept in SBUF) ----
        q_tiles = []
        for t in range(NT):
            xq = xp.tile([128, 2, 128], BF16, tag="xq")
            nc.sync.dma_start(
                out=xq[:],
                in_=xT_own[:, bass.ts(t, 128)].rearrange("(c k) n -> k c n", c=2))
            pq = psB.tile([128, D], F32, space="PSUM", tag="p256")
            nc.tensor.matmul(out=pq[:], lhsT=xq[:, 0, :], rhs=wq_t[:, 0, :],
                             start=True, stop=False)
            nc.tensor.matmul(out=pq[:], lhsT=xq[:, 1, :], rhs=wq_t[:, 1, :],
                             start=False, stop=True)
            q_t = qp.tile([128, D], BF16, tag=f"q{t}")
            nc.vector.tensor_tensor(out=q_t[:], in0=pq[:], in1=bq_t[:],
                                    op=mybir.AluOpType.add)
            q_tiles.append(q_t)

        # ---- phase A: attention per 128-node tile ----
        for t in range(NT):
            q_t = q_tiles[t]
            pu_t = ap_.tile([128, L], I32, tag="put")
            nc.sync.dma_start(out=pu_t[:], in_=pu[bass.ts(t, 128), :])
            pe_t = ap_.tile([128, DEG], I32, tag="pet")
            nc.sync.dma_start(out=pe_t[:], in_=pe[bass.ts(t, 128), :])

            ctx_r = cp.tile([128, DEG, D], F32, tag="ctxr")
            z_r = cp.tile([128, DEG, H], F32, tag="zr")

            for d in range(DEG):
                kvr = gp.tile([128, CARD, 2 * D], BF16, tag="kvr")
                for c in range(CARD):
                    nc.gpsimd.indirect_dma_start(
                        out=kvr[:, c, :], out_offset=None, in_=kv_tab[:],
                        in_offset=bass.IndirectOffsetOnAxis(
                            ap=pu_t[:, d * CARD + c:d * CARD + c + 1], axis=0))
                ek_g = gp.tile([128, D], BF16, tag="ekg")
                nc.gpsimd.indirect_dma_start(
                    out=ek_g[:], out_offset=None, in_=ek_tab[:],
                    in_offset=bass.IndirectOffsetOnAxis(
                        ap=pe_t[:, d:d + 1], axis=0))

                # qek[p,h] = sum_d q[p,h,:] * ek[p,h,:]
                prode = ap_.tile([128, D], BF16, tag="prode")
                nc.vector.tensor_tensor(out=prode[:], in0=ek_g[:], in1=q_t[:],
                                        op=mybir.AluOpType.mult)
                qek = ap_.tile([128, H], F32, tag="qek")
                nc.vector.tensor_reduce(
                    out=qek[:], in_=prode[:].rearrange("p (h e) -> p h e", h=H),
                    axis=mybir.AxisListType.X, op=mybir.AluOpType.add)

                # s[p,c,h] = sum_e q[p,h,e] * K[p,c,h,e]  (+ qek)
                prodk = ap_.tile([128, CARD, D], BF16, tag="prodk")
                nc.vector.tensor_tensor(
                    out=prodk[:], in0=kvr[:, :, 0:D],
                    in1=q_t[:].unsqueeze(1).to_broadcast([128, CARD, D]),
                    op=mybir.AluOpType.mult)
                s_d = ap_.tile([128, CARD, H], F32, tag="sd")
                nc.vector.tensor_reduce(
                    out=s_d[:],
                    in_=prodk[:].rearrange("p c (h e) -> p c h e", h=H),
                    axis=mybir.AxisListType.X, op=mybir.AluOpType.add)
                nc.vector.tensor_tensor(
                    out=s_d[:], in0=s_d[:],
                    in1=qek[:].unsqueeze(1).to_broadcast([128, CARD, H]),
                    op=mybir.AluOpType.add)

                # w = exp(s), z[p,h] = sum_c w[p,c,h]
                w_d = ap_.tile([128, CARD, H], BF16, tag="wd")
                nc.scalar.activation(out=w_d[:], in_=s_d[:],
                                     func=mybir.ActivationFunctionType.Exp)
                nc.vector.tensor_reduce(
                    out=z_r[:, d, :], in_=w_d[:].transpose([0, 2, 1]),
                    axis=mybir.AxisListType.X, op=mybir.AluOpType.add)

                # ctx_r[p,d,:] = sum_c w[p,c,h] * V[p,c,h,e]
                wv = ap_.tile([128, CARD, D], BF16, tag="wv")
                nc.vector.tensor_tensor(
                    out=wv[:].rearrange("p c (h e) -> p c h e", h=H),
                    in0=kvr[:, :, D:2 * D].rearrange("p c (h e) -> p c h e", h=H),
                    in1=w_d[:].unsqueeze(3).to_broadcast([128, CARD, H, DH]),
                    op=mybir.AluOpType.mult)
                nc.vector.tensor_reduce(
                    out=ctx_r[:, d, :], in_=wv[:].transpose([0, 2, 1]),
                    axis=mybir.AxisListType.X, op=mybir.AluOpType.add)

            # combine rounds
            ctx = tp.tile([128, D], F32, tag="ctx")
            nc.vector.tensor_reduce(
                out=ctx[:], in_=ctx_r[:].transpose([0, 2, 1]),
                axis=mybir.AxisListType.X, op=mybir.AluOpType.add)
            zsum = ap_.tile([128, H], F32, tag="zsum")
            nc.vector.tensor_reduce(
                out=zsum[:], in_=z_r[:].transpose([0, 2, 1]),
                axis=mybir.AxisListType.X, op=mybir.AluOpType.add)
            zrec = ap_.tile([128, H], F32, tag="zrec")
            nc.vector.reciprocal(out=zrec[:], in_=zsum[:])
            ctxn = tp.tile([128, D], F32, tag="ctxn")
            nc.vector.tensor_tensor(
                out=ctxn[:].rearrange("p (h e) -> p h e", h=H),
                in0=ctx[:].rearrange("p (h e) -> p h e", h=H),
                in1=zrec[:].unsqueeze(2).to_broadcast([128, H, DH]),
                op=mybir.AluOpType.mult)

            # out-proj: transpose ctxn, then PE matmul, bias, relu
            ctxT = tp.tile([128, 2, 128], F32, tag="ctxT")
            for ch in range(2):
                ptr = psC.tile([128, 128], F32, space="PSUM", tag="ptr")
                nc.tensor.transpose(out=ptr[:], in_=ctxn[:, bass.ts(ch, 128)],
                                    identity=ident[:])
                nc.scalar.copy(out=ctxT[:, ch, :], in_=ptr[:])
            po = psB.tile([128, D], F32, space="PSUM", tag="p256")
            nc.tensor.matmul(out=po[:], lhsT=ctxT[:, 0, :], rhs=owT_t[:, 0, :],
                             start=True, stop=False)
            nc.tensor.matmul(out=po[:], lhsT=ctxT[:, 1, :], rhs=owT_t[:, 1, :],
                             start=False, stop=True)
            ob = tp.tile([128, D], F32, tag="ob")
            nc.vector.tensor_tensor(out=ob[:], in0=po[:], in1=bo_t[:],
                                    op=mybir.AluOpType.add)
            o_sb = tp.tile([128, D], F32, tag="osb")
            nc.scalar.activation(out=o_sb[:], in_=ob[:],
                                 func=mybir.ActivationFunctionType.Relu)
            nc.sync.dma_start(out=out[bass.ts(t, 128), :], in_=o_sb[:])

    return nc


# ---------------------------------------------------------------------------
def host_prep(x, incidence, edge_attr, W_lin, W_edge,
              in_proj_w, in_proj_b, out_proj_w, out_proj_b):
    x = np.asarray(x, np.float32)
    inc = np.asarray(incidence, np.float32)
    ea = np.asarray(edge_attr, np.float32)
    W_lin = np.asarray(W_lin, np.float32)
    W_edge = np.asarray(W_edge, np.float32)
    in_proj_w = np.asarray(in_proj_w, np.float32)
    in_proj_b = np.asarray(in_proj_b, np.float32)
    out_proj_w = np.asarray(out_proj_w, np.float32)
    out_proj_b = np.asarray(out_proj_b, np.float32)

    # index lists from incidence (order within a node's pair set is irrelevant:
    # attention is permutation-invariant over the L pairs)
    eon = np.nonzero(inc.T)[1].reshape(N, DEG).astype(np.int32)   # edge_of_node
    noe = np.nonzero(inc)[1].reshape(E, CARD).astype(np.int32)    # node_of_edge
    pair_u = noe[eon].reshape(N, L).astype(np.int32)
    pair_e = eon

    Wq, Wk, Wv = in_proj_w[0:D], in_proj_w[D:2 * D], in_proj_w[2 * D:3 * D]
    bq, bk, bv = in_proj_b[0:D], in_proj_b[D:2 * D], in_proj_b[2 * D:3 * D]
    scale = 1.0 / np.sqrt(np.float32(DH))

    wkc = (W_lin @ Wk.T).astype(np.float32)
    wvc = (W_lin @ Wv.T).astype(np.float32)
    wqc = (W_lin @ Wq.T * scale).astype(np.float32)
    wek = (W_edge @ Wk.T).astype(np.float32)
    owT = out_proj_w.T.copy().astype(np.float32)

    import ml_dtypes
    bf = ml_dtypes.bfloat16
    rep = dict(
        xT=np.ascontiguousarray(x.T).astype(bf),
        eaT=np.ascontiguousarray(ea.T).astype(bf),
        wkc=wkc.astype(bf), wvc=wvc.astype(bf), wqc=wqc.astype(bf),
        wek=wek.astype(bf), owT=owT,
        bkv_b=np.broadcast_to(np.concatenate([np.zeros(D, np.float32), bv]),
                              (128, 2 * D)).copy(),
        bq_b=np.broadcast_to(bq * scale, (128, D)).copy(),
        bk_b=np.broadcast_to(bk, (128, D)).copy(),
        bo_b=np.broadcast_to(out_proj_b, (128, D)).copy(),
    )
    per_core = []
    for c in range(NCORES):
        sl = slice(c * NSH, (c + 1) * NSH)
        m = dict(rep)
        m["xT_own"] = np.ascontiguousarray(x.T[:, sl]).astype(bf)
        m["pu"] = pair_u[sl]
        m["pe"] = pair_e[sl]
        per_core.append(m)
    return per_core


_CACHE = {}


def kernel(x, incidence, edge_attr, W_lin, W_edge,
           in_proj_w, in_proj_b, out_proj_w, out_proj_b, deg, card):
    assert int(deg) == DEG and int(card) == CARD
    in_maps = host_prep(x, incidence, edge_attr, W_lin, W_edge,
                        in_proj_w, in_proj_b, out_proj_w, out_proj_b)
    if "nc" not in _CACHE:
        _CACHE["nc"] = build_nc()
    from concourse.bass_utils import run_bass_kernel_spmd
    res = run_bass_kernel_spmd(_CACHE["nc"], in_maps, list(range(NCORES)))
    return np.concatenate([res.results[c]["out"] for c in range(NCORES)], axis=0)



# revision 15
# speedup vs baseline: 2.6402x; 2.6402x over previous
"""Trainium2 Bass kernel for CustomHyperSemanticMessagePassing.

Hypergraph multi-head attention message passing, N=4096 nodes, E=4096 edges,
DEG=CARD=8, D=256, H=8 heads.

Sharding: ROUND-parallel. The regular hypergraph is 8 permutation rounds of
512 edges; each core processes one full round for ALL nodes. Within a round
every node belongs to exactly one edge, and laying positions out in permuted
(edge-block) order makes each edge's 8 members contiguous, so:

  - per 128-position tile, per head: scores are the block-diagonal of ONE
    PE matmul  S^T_h = kekT_h^T @ qT_h   (kek = k + ek of the shared edge,
    valid exactly on the block diagonal),
  - attention weights  wT_h = mask ⊙ exp(S^T_h)  (mask = 16x(8x8) blocks),
  - weighted values + softmax denominator are ONE PE matmul per head:
    [ctx_h | z_h] = wT_h^T @ [V_h | 1].

Per-round partials (ctx, z) are indirect-scattered to node order and summed
across cores (=rounds) with a single ReduceScatter; each core finishes its
own 512 nodes (normalize, out-proj, relu).

Exact identities used: key bias bk drops (softmax shift invariance); value
bias bv folds into the output bias (softmax weights sum to 1). q bias is
zero in this model. Softmax runs without max-subtraction (scores are O(1)).
"""
import numpy as np

import orjson
import concourse.bass as bass
import concourse.tile as tile
import concourse.bass_utils as bass_utils
import concourse.bass2jax as bass2jax
from concourse import mybir
from concourse.masks import make_identity
from concourse import library_config

F32 = mybir.dt.float32
BF16 = mybir.dt.bfloat16
I32 = mybir.dt.int32
I16 = mybir.dt.int16

N, E, D, EDGE_DIM = 4096, 4096, 256, 64
H, DH, DEG, CARD = 8, 32, 8, 8
NCORES = 8
NSH = N // NCORES            # nodes owned per core
NPOS = N                     # positions per core (one full round)
NT = NPOS // 128             # 128-position tiles per core
EPC = E // NCORES            # edges per round
SCH = 8                      # tiles per scatter chunk
PC = D + H                   # partial row: 8 heads x (32 ctx + 1 z)
HC = 3                       # head chunks (3 heads/chunk, 32 pad cols)


# ---------------------------------------------------------------------------
# walrus workaround: this build accepts only one sync-wait per instruction;
# split extras into injected single-wait NoOps at the BIR-JSON level.
_ORIG_COMPILE = bass_utils.compile_bir_kernel
_ctr = [0]


def _split_multiwaits(bir_json: bytes) -> bytes:
    bir = orjson.loads(bir_json)
    changed = False
    for f in bir.get("functions", []):
        for blk in f.get("blocks", []):
            out = []
            for ins in blk.get("instructions", []):
                si = ins.get("sync_info")
                waits = (si or {}).get("on_wait") or []
                if len(waits) > 1 and ins.get("engine") not in (None, "Unassigned"):
                    changed = True
                    for w in waits[:-1]:
                        _ctr[0] += 1
                        out.append({
                            "debug": ins.get("debug"),
                            "engine": ins["engine"],
                            "ins": [], "outs": [],
                            "name": f"WSPLIT-{_ctr[0]}",
                            "opcode": "NoOp",
                            "sync_info": {"on_wait": [w], "on_update": []},
                        })
                    si["on_wait"] = waits[-1:]
                out.append(ins)
            blk["instructions"] = out
    return orjson.dumps(bir) if changed else bir_json


def _patched_compile(bir_json, tmpdir, neff_name="file.neff"):
    return _ORIG_COMPILE(_split_multiwaits(bytes(bir_json)), tmpdir,
                         neff_name=neff_name)


bass_utils.compile_bir_kernel = _patched_compile
bass2jax.compile_bir_kernel = _patched_compile


# ---------------------------------------------------------------------------
def build_nc():
    nc = bass.Bass(num_devices=NCORES)
    # replicated inputs.  K and Q projections use a padded head layout of
    # HC chunks x (3 heads + 32 zero cols) so every head's 32 rows start at
    # partition 0/32/64 (PE stationary base-partition constraint).
    x_hbm = nc.declare_dram_parameter("x_bf", [N, D], BF16, isOutput=False)
    wk_p = nc.declare_dram_parameter("wk_p", [128, 2, HC, 128], BF16,
                                     isOutput=False)
    wv_p = nc.declare_dram_parameter("wv_p", [128, 2, D], BF16, isOutput=False)
    wq_p = nc.declare_dram_parameter("wq_p", [128, 2, HC, 128], BF16,
                                     isOutput=False)
    wek_p = nc.declare_dram_parameter("wek_p", [EDGE_DIM, HC, 128], BF16,
                                      isOutput=False)
    owt_p = nc.declare_dram_parameter("owt_p", [128, 2, D], BF16,
                                      isOutput=False)
    mask_p = nc.declare_dram_parameter("mask_p", [128, 128], BF16,
                                       isOutput=False)
    bo2_p = nc.declare_dram_parameter("bo2_p", [128, D], F32, isOutput=False)
    # per-core inputs
    gidx_p = nc.declare_dram_parameter("gidx", [128, 2, 128], I16,
                                       isOutput=False)
    scat_p = nc.declare_dram_parameter("scat", [128, NT], I32, isOutput=False)
    ea_p = nc.declare_dram_parameter("ea_own", [EDGE_DIM, EPC], BF16,
                                     isOutput=False)
    # output: this core's own nodes
    out_p = nc.declare_dram_parameter("out", [NSH, D], F32, isOutput=True)

    with tile.TileContext(nc) as tc, \
         tc.tile_pool(name="wp", bufs=1) as wp, \
         tc.tile_pool(name="xg", bufs=1) as xg, \
         tc.tile_pool(name="kp", bufs=3) as kp, \
         tc.tile_pool(name="vp", bufs=3) as vp, \
         tc.tile_pool(name="wtp", bufs=3) as wtp, \
         tc.tile_pool(name="stp", bufs=2) as stp, \
         tc.tile_pool(name="fp", bufs=2) as fp, \
         tc.tile_pool(name="psA", bufs=1, space="PSUM") as psA, \
         tc.tile_pool(name="psS", bufs=2, space="PSUM") as psS, \
         tc.tile_pool(name="psC", bufs=1, space="PSUM") as psC, \
         tc.tile_pool(name="psT", bufs=1, space="PSUM") as psT, \
         tc.tile_pool(name="dram", bufs=1, space="DRAM") as dram:

        # ---- resident weights / constants ----
        wk_t = wp.tile([128, 2, HC, 128], BF16)
        nc.sync.dma_start(out=wk_t[:], in_=wk_p[:])
        wv_t = wp.tile([128, 2, D], BF16)
        nc.sync.dma_start(out=wv_t[:], in_=wv_p[:])
        wq_t = wp.tile([128, 2, HC, 128], BF16)
        nc.sync.dma_start(out=wq_t[:], in_=wq_p[:])
        wek_t = wp.tile([EDGE_DIM, HC, 128], BF16)
        nc.sync.dma_start(out=wek_t[:], in_=wek_p[:])
        owt_t = wp.tile([128, 2, D], BF16)
        nc.sync.dma_start(out=owt_t[:], in_=owt_p[:])
        mask_t = wp.tile([128, 128], BF16)
        nc.sync.dma_start(out=mask_t[:], in_=mask_p[:])
        bo2_t = wp.tile([128, D], F32)
        nc.sync.dma_start(out=bo2_t[:], in_=bo2_p[:])
        gidx_t = wp.tile([128, 2, 128], I16)
        nc.sync.dma_start(out=gidx_t[:], in_=gidx_p[:])
        scat_t = wp.tile([128, NT], I32)
        nc.sync.dma_start(out=scat_t[:], in_=scat_p[:])
        ea_t = wp.tile([EDGE_DIM, EPC], BF16)
        nc.sync.dma_start(out=ea_t[:], in_=ea_p[:])
        ident = wp.tile([128, 128], BF16)
        make_identity(nc, ident[:])
        ones_t = wp.tile([128, 1], BF16)
        nc.vector.memset(ones_t[:], 1.0)
        nc.gpsimd.load_library(library_config.mlp)

        # ---- gather x rows transposed, in permuted (edge-block) order ----
        # out[c, k, i] = x[perm[i], k*128+c]  -> xT chunks per position
        xt_half = []
        for g in range(2):
            xt = xg.tile([128, 2, NPOS // 2], BF16)
            nc.gpsimd.dma_gather(
                out_ap=xt[:], in_ap=x_hbm[:], idxs_ap=gidx_t[:, g, :],
                num_idxs=NPOS // 2, num_idxs_reg=NPOS // 2, elem_size=D,
                transpose=True)
            xt_half.append(xt)

        # ---- ekT table for this round's edges: [d, e] = (ea @ Wek)^T ----
        # padded head layout, built in two PSUM pieces (chunks 0-1, chunk 2)
        ek_t = wp.tile([128, HC, EPC], BF16)
        pekA = psS.tile([128, H, 128], F32, tag="S")
        pekA_v = pekA[:].rearrange("p (c x) e -> p c (x e)", c=2)
        for ch in range(2):
            nc.tensor.matmul(out=pekA_v[:, ch, :],
                             lhsT=wek_t[:, ch, :], rhs=ea_t[:],
                             start=True, stop=True)
        nc.vector.tensor_copy(ek_t[:, 0:2, :], pekA_v[:])
        pekB = psS.tile([128, H, 128], F32, tag="S")
        pekB_v = pekB[:].rearrange("p (c x) e -> p c (x e)", c=2)
        nc.tensor.matmul(out=pekB_v[:, 0, :],
                         lhsT=wek_t[:, 2, :], rhs=ea_t[:],
                         start=True, stop=True)
        nc.vector.tensor_copy(ek_t[:, 2, :], pekB_v[:, 0, :])

        # ---- internal DRAM: per-round partials (node order) + RS output ----
        part_t = dram.tile([N, PC], F32)
        rs_t = dram.tile([NSH, PC], F32)

        # ---- main loop over 128-position tiles ----
        stage = None
        for t in range(NT):
            xt = xt_half[t // (NT // 2)][:, :, bass.ts(t % (NT // 2), 128)]

            # in-projections: kekT/qT (d on partitions) and v (pos on parts)
            pkq = psA.tile([128, 8, 128], F32, tag="pkqv")
            for ch in range(HC):
                for kc in range(2):
                    nc.tensor.matmul(out=pkq[:, ch, :],
                                     lhsT=wk_t[:, kc, ch, :],
                                     rhs=xt[:, kc, :],
                                     start=(kc == 0), stop=(kc == 1))
            for ch in range(HC):
                for kc in range(2):
                    nc.tensor.matmul(out=pkq[:, HC + ch, :],
                                     lhsT=wq_t[:, kc, ch, :],
                                     rhs=xt[:, kc, :],
                                     start=(kc == 0), stop=(kc == 1))
            pv = pkq[:, 6:8, :].rearrange("p a b -> p (a b)")
            for kc in range(2):
                nc.tensor.matmul(out=pv, lhsT=xt[:, kc, :],
                                 rhs=wv_t[:, kc, :],
                                 start=(kc == 0), stop=(kc == 1))

            # kek = k + ek (edge of each position's block), to bf16 SBUF
            kek_sb = kp.tile([128, HC, 128], BF16, tag="kek")
            nc.vector.tensor_tensor(
                out=kek_sb[:].rearrange("p c (b j) -> p c b j", b=16),
                in0=pkq[:, 0:HC, :].rearrange("p c (b j) -> p c b j", b=16),
                in1=ek_t[:, :, bass.ts(t, 16)].unsqueeze(3)
                    .to_broadcast([128, HC, 16, CARD]),
                op=mybir.AluOpType.add)
            qt_sb = kp.tile([128, HC, 128], BF16, tag="qT")
            nc.gpsimd.tensor_copy(qt_sb[:], pkq[:, HC:2 * HC, :])
            va = vp.tile([128, H, DH], BF16, tag="vaug")
            nc.gpsimd.tensor_copy(va[:].rearrange("p h e -> p (h e)"), pv)

            # per-head block scores: S^T_h = kekT_h^T @ qT_h
            pS = psS.tile([128, H, 128], F32, tag="S")
            for h in range(H):
                po, ch = DH * (h % 3), h // 3
                nc.tensor.matmul(out=pS[:, h, :],
                                 lhsT=kek_sb[po:po + DH, ch, :],
                                 rhs=qt_sb[po:po + DH, ch, :],
                                 start=True, stop=True)

            # attention weights: wT = mask * exp(S^T)
            wT = wtp.tile([128, H, 128], BF16, tag="wT")
            nc.scalar.activation(out=wT[:], in_=pS[:],
                                 func=mybir.ActivationFunctionType.Exp)
            wTm = wtp.tile([128, H, 128], BF16, tag="wTm")
            nc.vector.tensor_tensor(
                out=wTm[:], in0=wT[:],
                in1=mask_t[:].unsqueeze(1).to_broadcast([128, H, 128]),
                op=mybir.AluOpType.mult)

            # per-head [ctx | z] = wT_h^T @ [V_h | 1]
            pctx = psC.tile([128, H, DH + 1], F32, tag="ctx")
            for h in range(H):
                nc.tensor.matmul(out=pctx[:, h, 0:DH], lhsT=wTm[:, h, :],
                                 rhs=va[:, h, :], start=True, stop=True)
                nc.tensor.matmul(out=pctx[:, h, DH:DH + 1], lhsT=wTm[:, h, :],
                                 rhs=ones_t[:], start=True, stop=True)

            # stage partial rows; scatter a chunk of SCH tiles to node order
            if t % SCH == 0:
                stage = stp.tile([128, SCH, PC], F32, tag="stage")
            pctx_v = pctx[:].rearrange("p h e -> p (h e)")
            nc.vector.tensor_copy(stage[:, t % SCH, 0:PC // 2],
                                  pctx_v[:, 0:PC // 2])
            nc.scalar.copy(out=stage[:, t % SCH, PC // 2:PC],
                           in_=pctx_v[:, PC // 2:PC])
            if t % SCH == SCH - 1:
                c0 = t - (SCH - 1)
                nc.gpsimd.indirect_dma_start(
                    out=part_t[:],
                    out_offset=bass.IndirectOffsetOnAxis(
                        ap=scat_t[:, c0:c0 + SCH], axis=0),
                    in_=stage[:], in_offset=None)

        # ---- combine rounds across cores; keep own node chunk ----
        nc.gpsimd.collective_compute(
            "ReduceScatter", mybir.AluOpType.add,
            replica_groups=[list(range(NCORES))],
            ins=[part_t.opt()], outs=[rs_t.opt()])

        # ---- finish own nodes: normalize, out-proj, bias, relu ----
        for ft in range(NSH // 128):
            ld = fp.tile([128, H, DH + 1], F32, tag="ld")
            nc.sync.dma_start(
                out=ld[:],
                in_=rs_t[bass.ts(ft, 128), :].rearrange(
                    "p (h e) -> p h e", h=H))
            zr = fp.tile([128, H], F32, tag="zr")
            nc.vector.reciprocal(zr[:], ld[:, :, DH])
            cn = fp.tile([128, H, DH], BF16, tag="cn")
            nc.vector.tensor_tensor(
                out=cn[:], in0=ld[:, :, 0:DH],
                in1=zr[:].unsqueeze(2).to_broadcast([128, H, DH]),
                op=mybir.AluOpType.mult)
            cnT = fp.tile([128, 2, 128], BF16, tag="cnT")
            for dc in range(2):
                ptr = psT.tile([128, 128], BF16, tag="tr")
                nc.tensor.transpose(
                    out=ptr[:],
                    in_=cn[:, 4 * dc:4 * dc + 4, :].rearrange(
                        "p a b -> p (a b)"),
                    identity=ident[:])
                nc.scalar.copy(out=cnT[:, dc, :], in_=ptr[:])
            po = psA.tile([128, 8, 128], F32, tag="pkqv")
            po_v = po[:, 0:2, :].rearrange("p a b -> p (a b)")
            for dc in range(2):
                nc.tensor.matmul(out=po_v, lhsT=cnT[:, dc, :],
                                 rhs=owt_t[:, dc, :],
                                 start=(dc == 0), stop=(dc == 1))
            ob = fp.tile([128, D], F32, tag="ob")
            nc.vector.tensor_tensor(out=ob[:], in0=po_v, in1=bo2_t[:],
                                    op=mybir.AluOpType.add)
            oo = fp.tile([128, D], F32, tag="oo")
            nc.scalar.activation(out=oo[:], in_=ob[:],
                                 func=mybir.ActivationFunctionType.Relu)
            nc.sync.dma_start(out=out_p[bass.ts(ft, 128), :], in_=oo[:])

    return nc


# ---------------------------------------------------------------------------
def host_prep(x, incidence, edge_attr, W_lin, W_edge,
              in_proj_w, in_proj_b, out_proj_w, out_proj_b):
    import ml_dtypes
    bf = ml_dtypes.bfloat16

    x = np.asarray(x, np.float32)
    inc = np.asarray(incidence, np.float32)
    ea = np.asarray(edge_attr, np.float32)
    W_lin = np.asarray(W_lin, np.float32)
    W_edge = np.asarray(W_edge, np.float32)
    in_proj_w = np.asarray(in_proj_w, np.float32)
    in_proj_b = np.asarray(in_proj_b, np.float32)
    out_proj_w = np.asarray(out_proj_w, np.float32)
    out_proj_b = np.asarray(out_proj_b, np.float32)

    # members per edge; rounds are contiguous blocks of EPC edges
    noe = np.nonzero(inc)[1].reshape(E, CARD).astype(np.int64)

    Wq, Wk, Wv = in_proj_w[0:D], in_proj_w[D:2 * D], in_proj_w[2 * D:3 * D]
    bq, bv = in_proj_b[0:D], in_proj_b[2 * D:3 * D]
    assert not np.any(bq), "nonzero q bias not supported by this kernel"
    scale = 1.0 / np.sqrt(np.float32(DH))

    wkc = W_lin @ Wk.T                     # [D, D]
    wvc = W_lin @ Wv.T
    wqc = W_lin @ Wq.T * scale
    wek = W_edge @ Wk.T                    # [EDGE_DIM, D]
    owt = out_proj_w.T.copy()              # [D, D]
    bo2 = out_proj_b + bv @ out_proj_w.T   # bv folds through (sum w = 1)

    def pack(w):  # [D, D] -> [128, 2, D] with [k, kc, :] = w[kc*128+k, :]
        return np.ascontiguousarray(
            w.reshape(2, 128, D).transpose(1, 0, 2)).astype(bf)

    def pad_heads(w):  # [in, D] -> [in, HC, 128]: 3 heads + 32 zeros / chunk
        k = w.shape[0]
        out = np.zeros((k, HC, 128), np.float32)
        for ch in range(HC):
            cols = w[:, 96 * ch:96 * (ch + 1)]
            out[ch * 0 + 0:, ch, 0:cols.shape[1]] = cols
        return out.astype(bf)

    def pack_heads(w):  # [D, D] -> [128, 2, HC, 128] (k-chunked + head-pad)
        p = pad_heads(w)  # [D, HC, 128]
        return np.ascontiguousarray(
            p.reshape(2, 128, HC, 128).transpose(1, 0, 2, 3)).astype(bf)

    mask = np.kron(np.eye(16, dtype=np.float32),
                   np.ones((CARD, CARD), np.float32)).astype(bf)

    rep = dict(
        x_bf=x.astype(bf),
        wk_p=pack_heads(wkc), wv_p=pack(wvc), wq_p=pack_heads(wqc),
        wek_p=pad_heads(wek), owt_p=pack(owt),
        mask_p=mask,
        bo2_p=np.broadcast_to(bo2, (128, D)).copy(),
    )
    per_core = []
    for c in range(NCORES):
        perm = noe[c * EPC:(c + 1) * EPC].reshape(-1)     # [NPOS]
        gidx = np.empty((128, 2, 128), np.int16)
        for g in range(2):
            idsw = perm[g * 2048:(g + 1) * 2048].reshape(128, 16).T  # [16,128]
            gidx[:, g, :] = np.tile(idsw, (8, 1)).astype(np.int16)
        scat = perm.reshape(NT, 128).T.astype(np.int32)   # [128, NT]
        m = dict(rep)
        m["gidx"] = gidx
        m["scat"] = np.ascontiguousarray(scat)
        m["ea_own"] = np.ascontiguousarray(
            ea[c * EPC:(c + 1) * EPC].T).astype(bf)
        per_core.append(m)
    return per_core


_CACHE = {}


def kernel(x, incidence, edge_attr, W_lin, W_edge,
           in_proj_w, in_proj_b, out_proj_w, out_proj_b, deg, card):
    assert int(deg) == DEG and int(card) == CARD
    in_maps = host_prep(x, incidence, edge_attr, W_lin, W_edge,
                        in_proj_w, in_proj_b, out_proj_w, out_proj_b)
    if "nc" not in _CACHE:
        _CACHE["nc"] = build_nc()
    from concourse.bass_utils import run_bass_kernel_spmd
    res = run_bass_kernel_spmd(_CACHE["nc"], in_maps, list(range(NCORES)))
    return np.concatenate([res.results[c]["out"] for c in range(NCORES)],
                          axis=0)
